# revision 4
# baseline (speedup 1.0000x reference)
"""FLASH-style gated local+linear attention block on 8 Trainium2 NeuronCores.

Sharding: data-parallel over (batch, seq-half): core c handles batch c//2,
tokens [(c%2)*2048, (c%2+1)*2048). The linear-attention context lin_kv
[128, 2048] is all-reduced between the two cores sharing a batch.

Compute: bf16 matmuls with fp32 PSUM accumulation; LN stats and the final
residual add in fp32.
"""

import math
import os
import sys
import types
import contextlib
import ctypes

import numpy as np
import ml_dtypes

sys.path.insert(0, "/opt/trn_rl_repo")

import concourse.bacc as bacc
import concourse.mybir as mybir
import concourse.tile as tile
from concourse import bass_utils

bf16 = ml_dtypes.bfloat16
F32 = mybir.dt.float32
BF = mybir.dt.bfloat16
AF = mybir.ActivationFunctionType
OP = mybir.AluOpType
AX = mybir.AxisListType

DIM = 1024
HID = 2048
QK = 128
G = 256
B, N = 4, 4096
NUM_BUCKETS = 32
MAX_DIST = 128
N_CORES = 8
TOK = (B * N) // N_CORES          # tokens per core = 2048
NGRP = TOK // G                   # groups per core = 8


def _axon_hookshim():
    """Register antenv.axon_hooks (NTFF profiling hook) if the image lacks it.

    Only used when BASS_TRACE=1; harmless otherwise."""
    if "antenv.axon_hooks" in sys.modules:
        return
    try:
        import antenv
    except ImportError:
        return
    mod = types.ModuleType("antenv.axon_hooks")
    holder = [None]

    def set_axon_ntff_profile_hook(h):
        holder[0] = h

    def _make_hook(so_path):
        try:
            lib = ctypes.CDLL(so_path)
        except OSError:
            return None
        if not hasattr(lib, "axon_start_nrt_profile"):
            return None
        lib.axon_start_nrt_profile.argtypes = [ctypes.POINTER(ctypes.c_int64), ctypes.c_size_t]
        lib.axon_start_nrt_profile.restype = ctypes.c_int64
        lib.axon_stop_nrt_profile.argtypes = [ctypes.c_char_p]
        lib.axon_stop_nrt_profile.restype = ctypes.c_int64

        @contextlib.contextmanager
        def _hook(output_dir, device_ids):
            import jax
            jax.devices()
            if device_ids:
                ids = (ctypes.c_int64 * len(device_ids))(*device_ids)
                rc = lib.axon_start_nrt_profile(ids, len(device_ids))
            else:
                rc = lib.axon_start_nrt_profile(None, 0)
            if rc != 0:
                raise RuntimeError(f"axon_start_nrt_profile rc={rc}")
            try:
                yield
            finally:
                n = lib.axon_stop_nrt_profile(str(output_dir).encode())
                print(f"[kernel] profile files written: {n} -> {output_dir}", file=sys.stderr)

        return _hook

    def get_axon_ntff_profile_hook():
        if holder[0] is None:
            holder[0] = _make_hook("/opt/axon/libaxon_pjrt.so")
        return holder[0]

    mod.set_axon_ntff_profile_hook = set_axon_ntff_profile_hook
    mod.get_axon_ntff_profile_hook = get_axon_ntff_profile_hook
    sys.modules["antenv.axon_hooks"] = mod
    antenv.axon_hooks = mod


def _rel_pos_bias_T(rel_emb, scale):
    """biasT[k_pos, q_pos] of the reference's T5 bucket bias, fp32."""
    pos = np.arange(G)
    rel = pos[None, :] - pos[:, None]            # rel[q, k] = k - q
    n = -rel                                     # q - k
    nb = NUM_BUCKETS // 2                        # 16
    ret = (n < 0).astype(np.int32) * nb
    n = np.abs(n)
    max_exact = nb // 2                          # 8
    is_small = n < max_exact
    t = np.log(np.maximum(n, 1).astype(np.float32) / np.float32(max_exact))
    t = t / np.float32(math.log(MAX_DIST / max_exact)) * np.float32(nb - max_exact)
    val_large = max_exact + t.astype(np.int32)
    val_large = np.minimum(val_large, nb - 1)
    bucket = ret + np.where(is_small, n, val_large)      # [G, G] in [0, 32)
    bias = rel_emb[bucket, 0].astype(np.float32) * np.float32(scale)
    return np.ascontiguousarray(bias.T)


def _build(act=AF.Silu, phase1_only=False, skip_collective=False):
    nc = bacc.Bacc("TRN2", target_bir_lowering=False, debug=False, num_devices=N_CORES)

    # ---- external I/O (per core) ----
    xin = nc.dram_tensor("xin", [TOK, DIM], F32, kind="ExternalInput")
    wv_d = nc.dram_tensor("wv", [DIM, HID], BF, kind="ExternalInput")
    wg_d = nc.dram_tensor("wg", [DIM, HID], BF, kind="ExternalInput")
    wo_d = nc.dram_tensor("wo", [HID, DIM], BF, kind="ExternalInput")
    wqk_d = nc.dram_tensor("wqk", [DIM, QK], BF, kind="ExternalInput")
    bqk_d = nc.dram_tensor("bqk", [QK, 1], F32, kind="ExternalInput")
    bv_d = nc.dram_tensor("bv", [1, HID], BF, kind="ExternalInput")
    bgc_d = nc.dram_tensor("bgc", [128, 16], F32, kind="ExternalInput")
    bo_d = nc.dram_tensor("bo", [1, DIM], BF, kind="ExternalInput")
    osc_d = nc.dram_tensor("osc", [QK, 8], F32, kind="ExternalInput")
    biasT_d = nc.dram_tensor("biasT", [G, G], F32, kind="ExternalInput")
    out_d = nc.dram_tensor("out", [TOK, DIM], F32, kind="ExternalOutput")

    # ---- internal DRAM ----
    quad_d = nc.dram_tensor("quad_d", [NGRP, 128, 16 * 256], BF)
    gate_d = nc.dram_tensor("gate_d", [NGRP, 128, 16 * 256], BF)
    cc_in = nc.dram_tensor("cc_in", [128, HID], F32)
    cc_out = nc.dram_tensor("cc_out", [128, HID], F32)

    with tile.TileContext(nc) as tc:
        with tc.tile_pool(name="res", bufs=1) as res, \
             tc.tile_pool(name="wk", bufs=2) as wk, \
             tc.tile_pool(name="ps", bufs=4, space="PSUM") as ps, \
             tc.tile_pool(name="pskv", bufs=1, space="PSUM") as pskv:

            # ---- resident weights / constants ----
            Wv = res.tile([128, 8, HID], BF, name="Wv")
            nc.sync.dma_start(out=Wv, in_=wv_d.ap().rearrange("(c p) e -> p c e", p=128))
            Wg = res.tile([128, 8, HID], BF, name="Wg")
            nc.sync.dma_start(out=Wg, in_=wg_d.ap().rearrange("(c p) e -> p c e", p=128))
            Wo = res.tile([128, 16, DIM], BF, name="Wo")
            nc.sync.dma_start(out=Wo, in_=wo_d.ap().rearrange("(c p) e -> p c e", p=128))
            Wqk = res.tile([128, 8, QK], BF, name="Wqk")
            nc.sync.dma_start(out=Wqk, in_=wqk_d.ap().rearrange("(c p) e -> p c e", p=128))
            biasT = res.tile([128, 2, G], F32, name="biasT")
            nc.sync.dma_start(out=biasT, in_=biasT_d.ap().rearrange("(j p) i -> p j i", p=128))
            bqk = res.tile([128, 1], F32, name="bqk")
            nc.sync.dma_start(out=bqk, in_=bqk_d[:, :])
            bv = res.tile([1, HID], BF, name="bv")
            nc.sync.dma_start(out=bv, in_=bv_d[:, :])
            bgc = res.tile([128, 16], F32, name="bgc")
            nc.sync.dma_start(out=bgc, in_=bgc_d[:, :])
            bo = res.tile([1, DIM], BF, name="bo")
            nc.sync.dma_start(out=bo, in_=bo_d[:, :])
            osc = res.tile([128, 8], F32, name="osc")
            nc.sync.dma_start(out=osc, in_=osc_d[:, :])
            ones = res.tile([1, 128], BF, name="ones")
            nc.vector.memset(ones[:], 1.0)
            eps = res.tile([128, 1], F32, name="eps")
            nc.vector.memset(eps[:], 1e-5)
            linq_all = res.tile([128, NGRP, G], BF, name="linq_all")
            linkv_bf = res.tile([128, HID], BF, name="linkv_bf")

            # lin_kv accumulator: 4 PSUM banks, alive through phase 1
            pkv = pskv.tile([128, HID], F32, name="pkv")

            # =============== PHASE 1 ===============
            for g in range(NGRP):
                tok0 = g * G
                # nT[p, tb, c, t] = normed[tb*128 + t, c*128 + p]
                nT = wk.tile([128, 2, 8, 128], BF, name="nT")
                for tb in range(2):
                    xt = wk.tile([128, DIM], F32, name="xt")
                    nc.sync.dma_start(out=xt, in_=xin[tok0 + tb * 128: tok0 + (tb + 1) * 128, :])
                    # LN stats (fp32)
                    xsum = wk.tile([128, 1], F32, name="xsum")
                    nc.vector.reduce_sum(xsum[:], xt[:], axis=AX.X)
                    normed = wk.tile([128, DIM], BF, name="normed")
                    sqs = wk.tile([128, 1], F32, name="sqs")
                    # Square output is scratch (overwritten below); accum is what we need
                    nc.scalar.activation(normed[:], xt[:], AF.Square, accum_out=sqs[:])
                    mu = wk.tile([128, 1], F32, name="mu")
                    nc.vector.tensor_scalar(out=mu[:], in0=xsum[:], scalar1=1.0 / DIM,
                                            scalar2=None, op0=OP.mult)
                    ex2 = wk.tile([128, 1], F32, name="ex2")
                    nc.vector.tensor_scalar(out=ex2[:], in0=sqs[:], scalar1=1.0 / DIM,
                                            scalar2=None, op0=OP.mult)
                    negv = wk.tile([128, 1], F32, name="negv")
                    nc.vector.scalar_tensor_tensor(out=negv[:], in0=mu[:], scalar=mu[:],
                                                   in1=ex2[:], op0=OP.mult, op1=OP.subtract)
                    std = wk.tile([128, 1], F32, name="std")
                    nc.scalar.activation(std[:], negv[:], AF.Sqrt, bias=eps[:], scale=-1.0)
                    rstd = wk.tile([128, 1], F32, name="rstd")
                    nc.vector.reciprocal(rstd[:], std[:])
                    nmr = wk.tile([128, 1], F32, name="nmr")
                    nc.vector.tensor_scalar(out=nmr[:], in0=mu[:], scalar1=rstd[:],
                                            scalar2=-1.0, op0=OP.mult, op1=OP.mult)
                    nc.scalar.activation(normed[:], xt[:], AF.Identity, bias=nmr[:], scale=rstd[:])
                    nc.sync.dma_start_transpose(out=nT[:, tb], in_=normed[:])

                # qk^T [qk_dim, 256] = Wqk^T @ normed^T
                pqk = ps.tile([128, G], F32, name="pmm")
                for c in range(8):
                    nc.tensor.matmul(pqk[:], Wqk[:, c, :], nT[:, :, c, :],
                                     start=(c == 0), stop=(c == 7))

                # v [256, 2048] token-major
                v_sb = wk.tile([128, 2, 4, 512], BF, name="v_sb")
                for tb in range(2):
                    for ct in range(4):
                        pv = ps.tile([128, 512], F32, name="pmm")
                        for c in range(8):
                            nc.tensor.matmul(pv[:], nT[:, tb, c, :],
                                             Wv[:, c, ct * 512:(ct + 1) * 512],
                                             start=(c == 0), stop=False)
                        nc.tensor.matmul(pv[:], ones[:], bv[:, ct * 512:(ct + 1) * 512],
                                         start=False, stop=True)
                        nc.scalar.activation(v_sb[:, tb, ct, :], pv[:], act)

                # heads (silu + per-dim scale/offset), all [128, 256] dim-major
                qks = wk.tile([128, G], BF, name="qks")
                nc.scalar.activation(qks[:], pqk[:], act, bias=bqk[:])
                qq = wk.tile([128, G], BF, name="qq")
                nc.vector.tensor_scalar(out=qq[:], in0=qks[:], scalar1=osc[:, 0:1],
                                        scalar2=osc[:, 1:2], op0=OP.mult, op1=OP.add)
                qk2 = wk.tile([128, G], BF, name="qk2")
                nc.vector.tensor_scalar(out=qk2[:], in0=qks[:], scalar1=osc[:, 2:3],
                                        scalar2=osc[:, 3:4], op0=OP.mult, op1=OP.add)
                nc.vector.tensor_scalar(out=linq_all[:, g, :], in0=qks[:], scalar1=osc[:, 4:5],
                                        scalar2=osc[:, 5:6], op0=OP.mult, op1=OP.add)
                lkT = wk.tile([128, G], BF, name="lkT")
                nc.vector.tensor_scalar(out=lkT[:], in0=qks[:], scalar1=osc[:, 6:7],
                                        scalar2=osc[:, 7:8], op0=OP.mult, op1=OP.add)
                # lin_k to token-major [j, d] via xbar transpose
                lkt = wk.tile([128, 2, 128], BF, name="lkt")
                nc.sync.dma_start_transpose(out=lkt[:], in_=lkT[:])

                # attn^T = relu(sim^T + bias^T)^2, [j, i]
                attnT = wk.tile([128, 2, G], BF, name="attnT")
                for jb in range(2):
                    psim = ps.tile([128, G], F32, name="pmm")
                    nc.tensor.matmul(psim[:], qk2[:, jb * 128:(jb + 1) * 128], qq[:],
                                     start=True, stop=True)
                    nc.vector.tensor_tensor(out=psim[:], in0=psim[:], in1=biasT[:, jb, :], op=OP.add)
                    nc.vector.tensor_scalar(out=psim[:], in0=psim[:], scalar1=0.0,
                                            scalar2=None, op0=OP.max)
                    nc.scalar.activation(attnT[:, jb, :], psim[:], AF.Square)

                # gate^T [e, t] dim-major
                gateT = wk.tile([128, 16, G], BF, name="gateT")
                for eb in range(16):
                    pg = ps.tile([128, G], F32, name="pmm")
                    for c in range(8):
                        nc.tensor.matmul(pg[:], Wg[:, c, eb * 128:(eb + 1) * 128],
                                         nT[:, :, c, :], start=(c == 0), stop=(c == 7))
                    nc.scalar.activation(gateT[:, eb, :], pg[:], act, bias=bgc[:, eb:eb + 1])
                nc.sync.dma_start(out=gate_d[g], in_=gateT.rearrange("p a b -> p (a b)"))

                # quad_out^T [e, i]
                quadT = wk.tile([128, 16, G], BF, name="quadT")
                for eb in range(16):
                    pq = ps.tile([128, G], F32, name="pmm")
                    for jc in range(2):
                        nc.tensor.matmul(pq[:], v_sb[:, jc, eb // 4, (eb % 4) * 128:(eb % 4) * 128 + 128],
                                         attnT[:, jc, :], start=(jc == 0), stop=(jc == 1))
                    nc.scalar.copy(quadT[:, eb, :], pq[:])
                nc.sync.dma_start(out=quad_d[g], in_=quadT.rearrange("p a b -> p (a b)"))

                # lin_kv accumulation [d, e] over all groups
                for et in range(4):
                    for jc in range(2):
                        nc.tensor.matmul(pkv[:, et * 512:(et + 1) * 512], lkt[:, jc, :],
                                         v_sb[:, jc, et, :],
                                         start=(g == 0 and jc == 0),
                                         stop=(g == NGRP - 1 and jc == 1))

            # =============== all-reduce lin_kv within batch pair ===============
            linkv_f = wk.tile([128, HID], F32, name="v_sb")
            nc.vector.tensor_copy(linkv_f[:], pkv[:])
            nc.sync.dma_start(out=cc_in[:, :], in_=linkv_f[:])
            if phase1_only:
                nc.sync.dma_start(out=out_d[0:128, :], in_=linkv_f[:, 0:DIM])
            if skip_collective:
                nc.sync.dma_start(out=cc_out[:, :], in_=cc_in[:, :])
            else:
                nc.gpsimd.collective_compute(
                    "AllReduce", OP.add,
                    replica_groups=[[0, 1], [2, 3], [4, 5], [6, 7]],
                    ins=[cc_in.ap().opt()],
                    outs=[cc_out.ap().opt()],
                )
            # load reduced lin_kv and cast f32 -> bf16 on DVE
            linkv_f2 = wk.tile([128, HID], F32, name="v_sb")
            nc.sync.dma_start(out=linkv_f2[:], in_=cc_out[:, :])
            nc.vector.tensor_copy(linkv_bf[:], linkv_f2[:])

            # =============== PHASE 2 ===============
            for g in ([] if phase1_only else range(NGRP)):
                tok0 = g * G
                quad_re = wk.tile([128, 16, G], BF, name="v_sb")
                nc.sync.dma_start(out=quad_re.rearrange("p a b -> p (a b)"), in_=quad_d[g])
                gate_re = wk.tile([128, 16, G], BF, name="gateT")
                nc.sync.dma_start(out=gate_re.rearrange("p a b -> p (a b)"), in_=gate_d[g])

                outT = wk.tile([128, 16, G], BF, name="quadT")
                for eb in range(16):
                    plo = ps.tile([128, G], F32, name="pmm")
                    nc.tensor.matmul(plo[:], linkv_bf[:, eb * 128:(eb + 1) * 128],
                                     linq_all[:, g, :], start=True, stop=True)
                    t1 = wk.tile([128, G], F32, name="t1")
                    nc.vector.tensor_tensor(out=t1[:], in0=plo[:], in1=quad_re[:, eb, :], op=OP.add)
                    nc.vector.tensor_tensor(out=outT[:, eb, :], in0=t1[:], in1=gate_re[:, eb, :], op=OP.mult)

                for tb in range(2):
                    xr = wk.tile([128, DIM], F32, name="xt")
                    nc.sync.dma_start(out=xr, in_=xin[tok0 + tb * 128: tok0 + (tb + 1) * 128, :])
                    fin = wk.tile([128, DIM], F32, name="normed")
                    for ct in range(2):
                        pf = ps.tile([128, 512], F32, name="pmm")
                        for ec in range(16):
                            nc.tensor.matmul(pf[:], outT[:, ec, tb * 128:(tb + 1) * 128],
                                             Wo[:, ec, ct * 512:(ct + 1) * 512],
                                             start=(ec == 0), stop=False)
                        nc.tensor.matmul(pf[:], ones[:], bo[:, ct * 512:(ct + 1) * 512],
                                         start=False, stop=True)
                        nc.vector.tensor_tensor(out=fin[:, ct * 512:(ct + 1) * 512], in0=pf[:],
                                                in1=xr[:, ct * 512:(ct + 1) * 512], op=OP.add)
                    nc.sync.dma_start(out=out_d[tok0 + tb * 128: tok0 + (tb + 1) * 128, :], in_=fin[:])

    nc.compile()
    return nc


_NC_CACHE = None


def kernel(x, ln_g, ln_b, W_h, b_h, W_qk, b_qk, os_gamma, os_beta, rel_emb, W_out, b_out):
    global _NC_CACHE
    _axon_hookshim()

    x = np.asarray(x, dtype=np.float32)
    ln_g = np.asarray(ln_g, dtype=np.float32)
    ln_b = np.asarray(ln_b, dtype=np.float32)
    W_h = np.asarray(W_h, dtype=np.float32)
    b_h = np.asarray(b_h, dtype=np.float32)
    W_qk = np.asarray(W_qk, dtype=np.float32)
    b_qk = np.asarray(b_qk, dtype=np.float32)
    os_gamma = np.asarray(os_gamma, dtype=np.float32)
    os_beta = np.asarray(os_beta, dtype=np.float32)
    rel_emb = np.asarray(rel_emb, dtype=np.float32)
    W_out = np.asarray(W_out, dtype=np.float32)
    b_out = np.asarray(b_out, dtype=np.float32)

    # ---- CPU-side folding ----
    W_h_eff = ln_g[:, None] * W_h                    # layernorm gain folded into weights
    bh_eff = b_h + ln_b @ W_h                        # layernorm bias folded into bias
    wv = np.ascontiguousarray(W_h_eff[:, :HID]).astype(bf16)
    wg = np.ascontiguousarray(W_h_eff[:, HID:]).astype(bf16)
    wqk = (ln_g[:, None] * W_qk).astype(bf16)
    bqk = (b_qk + ln_b @ W_qk).astype(np.float32).reshape(QK, 1)
    bvv = bh_eff[:HID].astype(bf16).reshape(1, HID)
    bgc = np.ascontiguousarray(bh_eff[HID:].astype(np.float32).reshape(16, 128).T)
    wo = W_out.astype(bf16)
    bo = b_out.astype(bf16).reshape(1, DIM)
    # head scale/offsets: 0=quad_q (fold 1/G), 2=quad_k, 1=lin_q, 3=lin_k (fold 1/N)
    osc = np.stack([
        os_gamma[0] / G, os_beta[0] / G,
        os_gamma[2], os_beta[2],
        os_gamma[1], os_beta[1],
        os_gamma[3] / N, os_beta[3] / N,
    ], axis=1).astype(np.float32)                    # [128, 8]
    biasT = _rel_pos_bias_T(rel_emb, QK ** 0.5)

    shared = {
        "wv": wv, "wg": wg, "wo": wo, "wqk": wqk, "bqk": bqk, "bv": bvv,
        "bgc": bgc, "bo": bo, "osc": osc, "biasT": biasT,
    }
    xr = x.reshape(B, N, DIM)
    in_maps = []
    for c in range(N_CORES):
        b_idx, h_idx = c // 2, c % 2
        xc = np.ascontiguousarray(xr[b_idx, h_idx * TOK:(h_idx + 1) * TOK, :])
        in_maps.append({"xin": xc, **shared})

    if _NC_CACHE is None:
        _NC_CACHE = _build()
    nc = _NC_CACHE

    r = bass_utils.run_bass_kernel_spmd(nc, in_maps, core_ids=list(range(N_CORES)))
    kernel.last_results = r

    out = np.empty((B, N, DIM), dtype=np.float32)
    for c in range(N_CORES):
        b_idx, h_idx = c // 2, c % 2
        out[b_idx, h_idx * TOK:(h_idx + 1) * TOK, :] = r.results[c]["out"]
    return out
